# revision 8
# baseline (speedup 1.0000x reference)
"""Trainium2 Bass kernel v2 for the 4-layer linear-attention transformer.

Sharding: 8 cores; core c handles batch n=c//2, token half c%2 (T=8192).
Per layer the compacted KV/Ksum state (33KB) is AllReduce-summed within
core pairs; everything else local.

v2 design vs baseline:
- all matmuls bf16 (1 cyc/row at any free size; fp32r pays 4x at N<256)
- Q kept resident in SBUF bf16 (no DRAM spill / reload)
- attention + out-projection fused: out = Qz @ G with G = KVblk @ Wo^T,
  Qz = Q * Zb, Zb = 1/denom broadcast via block matmul (denomB = B^T Q)
- residual adds ride the PE via identity-matmul accumulation
- LN rstd via Newton iteration on DVE (no scalar-engine Sqrt ->
  scalar engine stays on one activation table: exp/copy/identity/relu)
- collective overlapped with the whole q-projection phase
- x canonical + output in bf16 (host upcasts), halves DMA + SBUF
"""

import numpy as np
import os
import sys
import contextlib

if "/opt/trn_rl_repo" not in sys.path:
    sys.path.insert(0, "/opt/trn_rl_repo")

import ml_dtypes
import concourse.bass as bass
import concourse.tile as tile
from concourse import mybir
from concourse.bass_test_utils import run_kernel

C = 256
F = 512
NL = 4
EPS_LN = 1e-5
N_CORES = 8
T_FULL = 16384
T = T_FULL // 2

F32 = mybir.dt.float32
F32R = mybir.dt.float32r
BF = mybir.dt.bfloat16
AF = mybir.ActivationFunctionType
ALU = mybir.AluOpType
BF_NP = ml_dtypes.bfloat16


def replica_groups(n_cores):
    return [[2 * i, 2 * i + 1] for i in range(n_cores // 2)]


def r_(ap):
    return ap.bitcast(F32R)


def _ln_newton(nc, P, mvg, tag):
    """mvg [128, 4, 2] = (mean, var) per token tile. Returns (rstd, nmr)
    tiles [128, 4]: rstd ~= 1/sqrt(var+eps), nmr = -mean*rstd.
    Newton iteration, valid for var in ~[0.3, 2.0]."""
    y = P["stats"].tile([128, 4], F32, tag=f"y{tag}", name=f"y{tag}")
    a = P["stats"].tile([128, 4], F32, tag=f"a{tag}", name=f"a{tag}")
    b = P["stats"].tile([128, 4], F32, tag=f"b{tag}", name=f"b{tag}")
    nm = P["stats"].tile([128, 4], F32, tag=f"nm{tag}", name=f"nm{tag}")
    v = mvg[:, :, 1]
    nc.vector.tensor_scalar(out=y[:], in0=v, scalar1=-0.5, scalar2=1.5 - 0.5 * EPS_LN,
                            op0=ALU.mult, op1=ALU.add)
    for _ in range(2):
        nc.vector.tensor_tensor(out=a[:], in0=y[:], in1=y[:], op=ALU.mult)
        nc.vector.scalar_tensor_tensor(out=b[:], in0=v, scalar=-0.5, in1=a[:],
                                       op0=ALU.mult, op1=ALU.mult)
        nc.vector.scalar_tensor_tensor(out=y[:], in0=b[:], scalar=1.5, in1=y[:],
                                       op0=ALU.add, op1=ALU.mult)
    nc.vector.scalar_tensor_tensor(out=nm[:], in0=mvg[:, :, 0], scalar=-1.0,
                                   in1=y[:], op0=ALU.mult, op1=ALU.mult)
    return y, nm


def emit_layer(tc, P, consts, ins, cur_x, l, T, out_y, n_cores):
    nc = tc.nc
    ntt = T // 128
    nch = T // 512

    ib128 = consts["ib128"]
    i128 = consts["i128"]

    # ---- weights (bf16) ----
    wq = [P["wts"].tile([128, 256], BF, tag=f"wq{i}", name=f"wq{i}") for i in range(2)]
    wkv = [P["wts"].tile([128, 512], BF, tag=f"wkv{i}", name=f"wkv{i}") for i in range(2)]
    woT = [P["wts"].tile([128, 256], BF, tag=f"wo{i}", name=f"wo{i}") for i in range(2)]
    w1 = [P["wts"].tile([128, 512], BF, tag=f"w1{i}", name=f"w1{i}") for i in range(2)]
    w2 = [P["wts"].tile([128, 256], BF, tag=f"w2{i}", name=f"w2{i}") for i in range(4)]
    for ci in range(2):
        nc.sync.dma_start(out=wq[ci][:], in_=ins["wqT"][l, ci * 128:(ci + 1) * 128, :])
        nc.sync.dma_start(out=wkv[ci][:], in_=ins["wkvT"][l, ci * 128:(ci + 1) * 128, :])
        nc.sync.dma_start(out=woT[ci][:], in_=ins["woT"][l, ci * 128:(ci + 1) * 128, :])
        nc.sync.dma_start(out=w1[ci][:], in_=ins["w1T"][l, ci * 128:(ci + 1) * 128, :])
    for ft in range(4):
        nc.sync.dma_start(out=w2[ft][:], in_=ins["w2T"][l, ft * 128:(ft + 1) * 128, :])
    bq0neg = P["wts"].tile([128, 2], F32, tag="bq0n", name="bq0n")
    nc.sync.dma_start(out=bq0neg[:], in_=ins["bq0n"][l])
    bq0 = P["wts"].tile([128, 2], F32, tag="bq0", name="bq0")
    bq1 = P["wts"].tile([128, 2], F32, tag="bq1", name="bq1")
    c1c = P["wts"].tile([128, 4], F32, tag="c1c", name="c1c")
    nc.sync.dma_start(out=bq0[:], in_=ins["bq0"][l])
    nc.sync.dma_start(out=bq1[:], in_=ins["bq1"][l])
    nc.sync.dma_start(out=c1c[:], in_=ins["c1c"][l])

    # resident per-layer: feature-major x (bf16) and Q (bf16)
    xf = P["xfres"].tile([128, 2, ntt, 128], BF, tag="xf", name="xf")
    Qres = P["qres"].tile([128, 2, nch, 512], BF, tag="Q", name="Q")

    # ---------------- phase 1a: transpose x, k/v proj, KV+Ksum accum ----
    kvacc = P["pskv"].tile([128, 258], F32, tag="kvacc", name="kvacc")
    # Clear-matmul: zero operands, start=True sets has_written for the whole
    # region; all accumulation chains below use start=False so no chain can
    # wipe another's bits (the bank is shared by 4 interleaved chains and
    # Tile may reorder them).
    nc.tensor.matmul(kvacc[:, 0:258], consts["zb128"], consts["z512"][:, 0:258],
                     start=True, stop=True)

    for ch in range(nch):
        for a2 in range(2):  # pairs of token tiles
            tpx = P["psA"].tile([128, 512], F32, tag="big", name="tpx")
            for t2 in range(2):
                i = ch * 4 + a2 * 2 + t2
                for ci in range(2):
                    off = ci * 256 + t2 * 128
                    nc.tensor.transpose(r_(tpx[:, off:off + 128]),
                                        r_(cur_x[i][:, ci * 128:(ci + 1) * 128]),
                                        r_(i128))
            nc.scalar.copy(out=xf[:, :, ch * 4 + a2 * 2: ch * 4 + a2 * 2 + 2, :],
                           in_=tpx[:])
        for tt in range(4):
            i = ch * 4 + tt
            kvp = P["psA"].tile([128, 512], F32, tag="big", name="kvp")
            nc.tensor.matmul(kvp[:], xf[:, 0, i, :], wkv[0][:], start=True, stop=False)
            nc.tensor.matmul(kvp[:], xf[:, 1, i, :], wkv[1][:], start=False, stop=True)
            rk = P["ektmp"].tile([128, 256], F32, tag="rk", name="rk")
            nc.scalar.activation(out=rk[:], in_=kvp[:, 0:256], func=AF.Relu,
                                 bias=0.0, scale=-1.0)
            ek = P["ektmp"].tile([128, 256], F32, tag="ek", name="ek")
            nc.scalar.activation(out=ek[:], in_=rk[:], func=AF.Exp,
                                 bias=0.0, scale=-1.0)
            ktt = P["kt"].tile([128, 256], BF, tag="kt", name="kt")
            nc.vector.scalar_tensor_tensor(out=ktt[:], in0=kvp[:, 0:256], scalar=1.0,
                                           in1=ek[:], op0=ALU.add, op1=ALU.max)
            vt = P["vt"].tile([128, 256], BF, tag="vt", name="vt")
            nc.scalar.copy(out=vt[:], in_=kvp[:, 256:512])
            sp = (i == ntt - 1)
            nc.tensor.matmul(kvacc[:, 0:128], ktt[:, 0:128], vt[:, 0:128],
                             start=False, stop=sp)
            nc.tensor.matmul(kvacc[:, 128:256], ktt[:, 128:256], vt[:, 128:256],
                             start=False, stop=sp)
            nc.tensor.matmul(kvacc[:, 256:257], ktt[:, 0:128], consts["onesb"],
                             start=False, stop=sp)
            nc.tensor.matmul(kvacc[:, 257:258], ktt[:, 128:256], consts["onesb"],
                             start=False, stop=sp)

    # ---------------- collective ----------------
    # compact [128,258] -> [128,66]: per half 4 diag blocks + ksum col
    kvc = P["small"].tile([128, 72], F32, tag="kvc", name="kvc")
    nc.vector.memset(kvc[:], 0.0)
    for half in range(2):
        base = half * 36
        for h in range(4):
            r0 = h * 32
            nc.vector.tensor_copy(out=kvc[r0:r0 + 32, base:base + 32],
                                  in_=kvacc[r0:r0 + 32, half * 128 + r0:half * 128 + r0 + 32])
        nc.vector.tensor_copy(out=kvc[:, base + 32:base + 33],
                              in_=kvacc[:, 256 + half:257 + half])
    ccin = P["dram"].tile([128, 72], F32, tag="ccin", name="ccin")
    ccout = P["dram"].tile([128, 72], F32, tag="ccout", name="ccout")
    nc.sync.dma_start(out=ccin[:], in_=kvc[:])
    nc.gpsimd.collective_compute(
        "AllReduce", ALU.add, replica_groups=replica_groups(n_cores),
        ins=[ccin[:].opt()], outs=[ccout[:].opt()])
    kvf = P["small"].tile([128, 72], F32, tag="kvf", name="kvf")
    nc.sync.dma_start(out=kvf[:], in_=ccout[:])

    # ---------------- phase 1b: q proj + feature map (overlaps collective)
    for ch in range(nch):
        for co in range(2):
            qp = P["psA"].tile([128, 512], F32, tag="big", name="qp")
            nc.tensor.matmul(qp[:], wq[0][:, co * 128:(co + 1) * 128],
                             xf[:, 0, ch * 4:(ch + 1) * 4, :], start=True, stop=False)
            nc.tensor.matmul(qp[:], wq[1][:, co * 128:(co + 1) * 128],
                             xf[:, 1, ch * 4:(ch + 1) * 4, :], start=False, stop=True)
            rq = P["etmp"].tile([128, 512], F32, tag="rq", name="rq")
            nc.scalar.activation(out=rq[:], in_=qp[:], func=AF.Relu,
                                 bias=bq0neg[:, co:co + 1], scale=-1.0)
            e = P["etmp"].tile([128, 512], F32, tag="e", name="e")
            nc.scalar.activation(out=e[:], in_=rq[:], func=AF.Exp,
                                 bias=0.0, scale=-1.0)
            nc.vector.scalar_tensor_tensor(out=Qres[:, co, ch, :], in0=qp[:],
                                           scalar=bq1[:, co:co + 1], in1=e[:],
                                           op0=ALU.add, op1=ALU.max)

    # ---------------- builds: kvblkT -> G, B ----------------
    G = []
    B = []
    for half in range(2):
        base = half * 36
        kb = P["small"].tile([128, 128], F32, tag=f"kb{half}", name=f"kb{half}")
        nc.scalar.activation(out=r_(kb[:]), in_=i128, func=AF.Copy, bias=0.0, scale=0.0)
        for h in range(4):
            r0 = h * 32
            nc.vector.tensor_copy(out=r_(kb[r0:r0 + 32, r0:r0 + 32]),
                                  in_=kvf[r0:r0 + 32, base:base + 32])
        kbt_ps = P["psA"].tile([128, 512], F32, tag="big", name="kbt")
        nc.tensor.transpose(r_(kbt_ps[:, 0:128]), r_(kb[:]), r_(i128))
        kbt = P["small"].tile([128, 128], BF, tag=f"kbt{half}", name=f"kbt{half}")
        nc.scalar.copy(out=kbt[:], in_=kbt_ps[:, 0:128])
        g_ps = P["psA"].tile([128, 512], F32, tag="big", name="gps")
        nc.tensor.matmul(g_ps[:, 0:256], kbt[:], woT[half][:], start=True, stop=True)
        g = P["small"].tile([128, 256], BF, tag=f"g{half}", name=f"g{half}")
        nc.scalar.copy(out=g[:], in_=g_ps[:, 0:256])
        G.append(g)
        bb = P["small"].tile([128, 128], BF, tag=f"bb{half}", name=f"bb{half}")
        nc.scalar.activation(out=bb[:], in_=i128, func=AF.Copy, bias=0.0, scale=0.0)
        for h in range(4):
            r0 = h * 32
            nc.scalar.activation(out=bb[r0:r0 + 32, r0:r0 + 32],
                                 in_=i128[r0:r0 + 32, 0:32], func=AF.Identity,
                                 bias=kvf[r0:r0 + 32, base + 32:base + 33], scale=0.0)
        B.append(bb)

    # ---------------- phase 2 ----------------
    new_x = []
    for ch in range(nch):
        zb = []
        for half in range(2):
            dn = P["psA"].tile([128, 512], F32, tag="big", name="dn")
            nc.tensor.matmul(dn[:], B[half][:], Qres[:, half, ch, :],
                             start=True, stop=True)
            z = P["zsb"].tile([128, 512], BF, tag=f"zb{half}", name=f"zb{half}")
            nc.vector.reciprocal(out=z[:], in_=dn[:])
            zb.append(z)
        qz = []
        for half in range(2):
            t = P["qztmp"].tile([128, 512], BF, tag=f"qz{half}", name=f"qz{half}")
            nc.vector.tensor_tensor(out=t[:], in0=Qres[:, half, ch, :], in1=zb[half][:],
                                    op=ALU.mult)
            qz.append(t)

        # fused o-proj + residual on PE; LN1 stats from PSUM
        mvg1 = P["stats"].tile([128, 4, 2], F32, tag="mvg1", name="mvg1")
        opps = []
        for tt in range(4):
            i = ch * 4 + tt
            if tt % 2 == 0:
                opp = P["psU"].tile([128, 512], F32, tag="u", name="opp")
                nc.tensor.matmul(opp[:], consts["zb128"], consts["z512"],
                                 start=True, stop=True)
                opps.append(opp)
            col = (tt % 2) * 256
            sl = opps[tt // 2][:, col:col + 256]
            nc.tensor.matmul(sl, qz[0][:, tt * 128:(tt + 1) * 128], G[0][:],
                             start=False, stop=False)
            nc.tensor.matmul(sl, qz[1][:, tt * 128:(tt + 1) * 128], G[1][:],
                             start=False, stop=False)
            nc.tensor.matmul(sl, r_(i128), r_(cur_x[i][:]), start=False, stop=True)
            st6 = P["stats"].tile([128, 6], F32, tag="st6", name="st6")
            nc.vector.bn_stats(out=st6[:], in_=sl)
            nc.vector.bn_aggr(out=mvg1[:, tt, :], in_=st6[:])
        rstd1, nm1 = _ln_newton(nc, P, mvg1, "1")

        x1_t = []
        for tt in range(4):
            x1t = P["x1p"].tile([128, 256], BF, tag="x1", name="x1")
            nc.scalar.activation(out=x1t[:], in_=opps[tt // 2][:, (tt % 2) * 256:(tt % 2) * 256 + 256],
                                 func=AF.Identity,
                                 bias=nm1[:, tt:tt + 1], scale=rstd1[:, tt:tt + 1])
            x1_t.append(x1t)

        # x1 -> feature-major bf16
        x1f = P["x1fp"].tile([128, 2, 4, 128], BF, tag="x1f", name="x1f")
        for a2 in range(2):
            tp = P["psU"].tile([128, 512], F32, tag="u", name="tp")
            tb = tp[:].bitcast(BF)
            for t2 in range(2):
                for ci in range(2):
                    off = ci * 256 + t2 * 128
                    nc.tensor.transpose(tb[:, off:off + 128],
                                        x1_t[a2 * 2 + t2][:, ci * 128:(ci + 1) * 128],
                                        ib128)
            nc.scalar.copy(out=x1f[:, :, a2 * 2:a2 * 2 + 2, :], in_=tb[:, 0:512])

        # FFN hidden
        hs_t = []
        for ft in range(4):
            hp = P["psU"].tile([128, 512], F32, tag="u", name="hp")
            nc.tensor.matmul(hp[:], w1[0][:, ft * 128:(ft + 1) * 128],
                             x1f[:, 0, :, :], start=True, stop=False)
            nc.tensor.matmul(hp[:], w1[1][:, ft * 128:(ft + 1) * 128],
                             x1f[:, 1, :, :], start=False, stop=True)
            hs = P["hfm"].tile([128, 512], BF, tag="hs", name="hs")
            if ft < 2:
                nc.scalar.activation(out=hs[:], in_=hp[:], func=AF.Relu,
                                     bias=c1c[:, ft:ft + 1], scale=1.0)
            else:
                nc.vector.tensor_scalar(out=hs[:], in0=hp[:],
                                        scalar1=c1c[:, ft:ft + 1], scalar2=0.0,
                                        op0=ALU.add, op1=ALU.max)
            hs_t.append(hs)

        # FFN out + residual on PE; LN2 stats
        mvg2 = P["stats"].tile([128, 4, 2], F32, tag="mvg2", name="mvg2")
        ypps = []
        for tt in range(4):
            if tt % 2 == 0:
                ypp = P["psU"].tile([128, 512], F32, tag="u", name="ypp")
                nc.tensor.matmul(ypp[:], consts["zb128"], consts["z512"],
                                 start=True, stop=True)
                ypps.append(ypp)
            col = (tt % 2) * 256
            sl = ypps[tt // 2][:, col:col + 256]
            for ft in range(4):
                nc.tensor.matmul(sl, hs_t[ft][:, tt * 128:(tt + 1) * 128], w2[ft][:],
                                 start=False, stop=False)
            nc.tensor.matmul(sl, ib128, x1_t[tt][:], start=False, stop=True)
            st6b = P["stats"].tile([128, 6], F32, tag="st6b", name="st6b")
            nc.vector.bn_stats(out=st6b[:], in_=sl)
            nc.vector.bn_aggr(out=mvg2[:, tt, :], in_=st6b[:])
        rstd2, nm2 = _ln_newton(nc, P, mvg2, "2")

        for tt in range(4):
            i = ch * 4 + tt
            x2t = P["xres"].tile([128, 256], F32, tag="xres", name="xres")
            nc.scalar.activation(out=r_(x2t[:]), in_=ypps[tt // 2][:, (tt % 2) * 256:(tt % 2) * 256 + 256],
                                 func=AF.Identity,
                                 bias=nm2[:, tt:tt + 1], scale=rstd2[:, tt:tt + 1])
            nc.sync.dma_start(out=out_y[l, i * 128:(i + 1) * 128, :], in_=x2t[:])
            new_x.append(x2t)

    return new_x


def kernel_body(tc, outs, ins, T, n_cores=N_CORES):
    nc = tc.nc
    ntt = T // 128

    ctx = contextlib.ExitStack()
    tc._kernel_ctx = ctx
    P = {}

    def pool(name, bufs, space="SBUF"):
        P[name] = ctx.enter_context(
            tc.tile_pool(name=name, bufs=bufs, space=space))

    # PSUM: 8 banks: psA(3) + psU(4, shared opp/tp/hp/ypp) + pskv(1)
    pool("psA", 3, space="PSUM")
    pool("psU", 4, space="PSUM")
    pool("pskv", 1, space="PSUM")
    # SBUF pools
    pool("xfres", 1)
    pool("qres", 1)
    pool("ektmp", 3)
    pool("etmp", 2)
    pool("kt", 3)
    pool("vt", 3)
    pool("zsb", 2)
    pool("qztmp", 2)
    pool("x1p", 8)
    pool("x1fp", 3)
    pool("hfm", 6)
    pool("stats", 4)
    pool("small", 2)
    pool("wts", 2)
    pool("consts", 1)
    pool("xres", ntt + 6)
    pool("dram", 2, space="DRAM")

    cp = P["consts"]
    i128 = cp.tile([128, 128], F32, tag="i128", name="i128")
    nc.sync.dma_start(out=r_(i128[:]), in_=r_(ins["i128"]))
    ib = cp.tile([128, 128], BF, tag="ib128", name="ib128")
    nc.sync.dma_start(out=ib[:], in_=ins["ib128"])
    onesb = cp.tile([128, 1], BF, tag="onesb", name="onesb")
    nc.sync.dma_start(out=onesb[:], in_=ins["onesb"])
    zb128 = cp.tile([128, 128], BF, tag="zb128", name="zb128")
    nc.vector.memset(zb128[:], 0.0)
    z512 = cp.tile([128, 512], BF, tag="z512", name="z512")
    nc.vector.memset(z512[:], 0.0)
    consts = {"i128": i128[:], "ib128": ib[:], "onesb": onesb[:],
              "zb128": zb128[:], "z512": z512[:]}

    cur_x = []
    for i in range(ntt):
        t = P["xres"].tile([128, 256], F32, tag="xres", name="xres")
        nc.sync.dma_start(out=r_(t[:]), in_=r_(ins["x0"][i * 128:(i + 1) * 128, :]))
        cur_x.append(t)

    out_y = outs["y"]
    with nc.allow_low_precision(reason="bf16 kernel by design; tolerance 2e-2"):
        for l in range(NL):
            cur_x = emit_layer(tc, P, consts, ins, cur_x, l, T, out_y, n_cores)

    ctx.close()


def prep_inputs(inputs, T, n_cores):
    rf = np.asarray(inputs["ref_feature"], np.float32)
    N = rf.shape[0]
    t_full = rf.shape[2] * rf.shape[3]
    x_tok = rf.reshape(N, C, t_full).transpose(0, 2, 1)

    for nm in ("bk", "bv", "bo", "c2", "be1", "be2"):
        assert not np.any(np.asarray(inputs[nm])), f"nonzero {nm} unsupported"
    for nm in ("g1", "g2"):
        assert np.all(np.asarray(inputs[nm]) == 1.0), f"non-unit {nm} unsupported"

    bf = BF_NP
    wqT = np.ascontiguousarray(np.asarray(inputs["Wq"]).transpose(0, 2, 1)).astype(bf)
    wkT = np.asarray(inputs["Wk"]).transpose(0, 2, 1)
    wvT = np.asarray(inputs["Wv"]).transpose(0, 2, 1)
    wkvT = np.ascontiguousarray(np.concatenate([wkT, wvT], axis=2)).astype(bf)
    woT = np.ascontiguousarray(np.asarray(inputs["Wo"]).transpose(0, 2, 1)).astype(bf)
    w1T = np.ascontiguousarray(np.asarray(inputs["W1"]).transpose(0, 2, 1)).astype(bf)
    w2T = np.ascontiguousarray(np.asarray(inputs["W2"]).transpose(0, 2, 1)).astype(bf)

    bq = np.asarray(inputs["bq"], np.float32)
    bq_col = np.ascontiguousarray(bq.reshape(NL, 2, 128).transpose(0, 2, 1))
    bq0n_col = np.ascontiguousarray((-bq).reshape(NL, 2, 128).transpose(0, 2, 1))
    bq1_col = np.ascontiguousarray((bq + 1.0).reshape(NL, 2, 128).transpose(0, 2, 1))
    c1 = np.asarray(inputs["c1"], np.float32)
    c1_col = np.ascontiguousarray(c1.reshape(NL, 4, 128).transpose(0, 2, 1))

    i128 = np.eye(128, dtype=np.float32)
    ib128 = np.eye(128, dtype=np.float32).astype(bf)
    onesb = np.ones((128, 1), np.float32).astype(bf)

    shared = dict(wqT=wqT, wkvT=wkvT, woT=woT, w1T=w1T, w2T=w2T,
                  bq0=bq_col, bq0n=bq0n_col, bq1=bq1_col, c1c=c1_col,
                  i128=i128, ib128=ib128, onesb=onesb)
    per_core = []
    halves = t_full // T
    for c in range(n_cores):
        n, half = c // halves, c % halves
        x0 = np.ascontiguousarray(x_tok[n, half * T:(half + 1) * T, :])
        d = dict(shared)
        d["x0"] = x0
        per_core.append(d)
    return per_core


def unshard_output(ys, N, Hh=128, Ww=128):
    """ys: per-core [NL, T, C] bf16 list -> [NL, N, C, H, W] fp32."""
    out = np.empty((NL, N, C, Hh, Ww), np.float32)
    rows_per_core = T // Ww
    for c, y in enumerate(ys):
        n, half = c // 2, c % 2
        row0 = half * rows_per_core
        yf = np.asarray(y, np.float32)
        for l in range(NL):
            blk = np.ascontiguousarray(yf[l]).T.reshape(C, rows_per_core, Ww)
            out[l, n, :, row0:row0 + rows_per_core, :] = blk
    return out


LAST_EXEC_NS = None
LAST_RES = None


def kernel(**inputs):
    per_core = prep_inputs(inputs, T, N_CORES)
    output_like = [dict(y=np.zeros((NL, T, C), np.float32))
                   for _ in range(N_CORES)]

    def body(tc, outs, ins):
        kernel_body(tc, outs, ins, T)

    trace_hw = bool(os.environ.get("TRACE_HW"))
    res = run_kernel(body, None, per_core, bass_type=tile.TileContext,
                     num_cores=N_CORES, check_with_sim=False,
                     check_with_hw=True, trace_hw=trace_hw,
                     output_like=output_like)
    global LAST_EXEC_NS, LAST_RES
    LAST_EXEC_NS = res.exec_time_ns
    LAST_RES = res
    rkey = list(res.results[0].keys())[0]
    ys = [r[rkey] for r in res.results]
    N = np.asarray(inputs["ref_feature"]).shape[0]
    return unshard_output(ys, N)
